# revision 20
# baseline (speedup 1.0000x reference)
"""GCNConv kernel for 8 TRN2 NeuronCores.

Computes: out = A_hat @ (X @ W + b)
  X: [16384, 512] f32   A_hat: [16384, 16384] f32
  W: [512, 256] f32     b: [256] f32          out: [16384, 256] f32

Sharding: row-shard A_hat / out across 8 cores (2048 rows each); replicate
X, W, b. Each core computes the full projection H = X @ W + b (SBUF-
resident) and then its slice of the aggregation A_rows @ H.

Host-side layout prep (sharding, not device work):
  - A shard is pre-transposed to AT = A[rows, :].T -> [16384, 2048] so the
    contraction dim lands on SBUF partitions with coalesced DMA loads, and
    converted to bf16 (halves HBM traffic; fp32 accumulation on device).
  - X is pre-transposed to XT = X.T -> [512, 16384] (bf16).
The device output is outT = (A_rows @ H).T [256, 2048]; the host
transposes back and concatenates. All accumulation is fp32 in PSUM.
"""

import numpy as np
import ml_dtypes

import concourse.bass as bass
import concourse.mybir as mybir
import concourse.tile as tile
from concourse import bacc
from concourse.bass_utils import run_bass_kernel_spmd

N = 16384
D_IN = 512
D_OUT = 256
N_CORES = 8
ROWS = N // N_CORES  # 2048 A/out rows per core

P = 128
F32 = mybir.dt.float32
F32R = mybir.dt.float32r
BF16 = mybir.dt.bfloat16

# compute dtype config: 'bf16' or 'f32r' for the A/X/W/H operand storage
A_DTYPE = "bf16"
X_DTYPE = "bf16"


def _dt(tag):
    return BF16 if tag == "bf16" else F32R


def _np_dt(tag):
    return ml_dtypes.bfloat16 if tag == "bf16" else np.float32


def build_gcn_nc(n=N, d_in=D_IN, d_out=D_OUT, rows=ROWS,
                 a_dtype=A_DTYPE, x_dtype=X_DTYPE, a_bufs=12, x_cols=512):
    """Per-core SPMD program.

    DRAM params (per core):
      AT  [n, rows]    a_dtype  - A shard, transposed
      XT  [d_in, n]    x_dtype  - X, transposed (replicated)
      W   [d_in, d_out] x_dtype
      b   [1, d_out]   f32
      outT [d_out, rows] f32 (output)
    """
    KB = n // P            # aggregation contraction blocks (128)
    DB = d_in // P         # projection contraction blocks (4)
    XC = min(x_cols, n)    # X columns loaded per DMA
    XB = n // XC           # X load blocks (32)
    XS = XC // P           # i-subblocks per X load (4)
    JH = d_out // P        # output-column halves (2)
    NC_F = min(512, rows)
    N_IC = rows // NC_F    # rhs chunks per aggregation step (4)

    adt = _dt(a_dtype)
    xdt = _dt(x_dtype)

    nc = bacc.Bacc("TRN2", target_bir_lowering=False, debug=False,
                   num_devices=N_CORES)

    AT = nc.dram_tensor("AT", [n, rows], adt, kind="ExternalInput").ap()
    XT = nc.dram_tensor("XT", [d_in, n], xdt, kind="ExternalInput").ap()
    W = nc.dram_tensor("W", [d_in, d_out], xdt, kind="ExternalInput").ap()
    b = nc.dram_tensor("b", [1, d_out], F32, kind="ExternalInput").ap()
    outT = nc.dram_tensor("outT", [d_out, rows], F32, kind="ExternalOutput").ap()

    XT_r = XT.rearrange("(a p) i -> p a i", p=P)   # [128, DB, n]
    W_r = W.rearrange("(a p) j -> p a j", p=P)     # [128, DB, d_out]

    with tile.TileContext(nc) as tc:
        with (
            tc.tile_pool(name="const", bufs=1) as const_pool,
            tc.tile_pool(name="hbuf", bufs=1) as h_pool,
            tc.tile_pool(name="xbuf", bufs=6) as x_pool,
            tc.tile_pool(name="abuf", bufs=a_bufs) as a_pool,
            tc.tile_pool(name="obuf", bufs=2) as o_pool,
            tc.tile_pool(name="psum", bufs=8, space="PSUM") as psum_pool,
        ):
            # ---- constants ----
            # per-k-block W tiles so the first matmul starts after 1/DB of
            # the W load
            w_blk = [const_pool.tile([P, d_out], xdt, name=f"w_blk{a}")
                     for a in range(DB)]
            for a in range(DB):
                nc.sync.dma_start(w_blk[a][:], W_r[:, a, :])
            b_sb = const_pool.tile([1, d_out], F32)
            nc.sync.dma_start(b_sb[:], b[:])
            b128 = const_pool.tile([P, d_out], F32)
            nc.gpsimd.partition_broadcast(b128[:], b_sb[:])

            # H = X @ W + b, stored as h_all[p, kb, j] = H[kb*128 + p, j]
            h_all = h_pool.tile([P, KB, d_out], adt)

            # ---- phase 1: projection ----
            # (first block loads narrow so the PE starts as early as possible)
            ramp = [P, 2 * P] if n >= 3 * P + XC else [P]
            blocks = []
            off = 0
            for w_ in ramp:
                blocks.append((off, w_)); off += w_
            while off + XC <= n:
                blocks.append((off, XC)); off += XC
            if off < n:
                blocks.append((off, n - off))
            for off, width in blocks:
                x_tile = x_pool.tile([P, DB, XC], xdt, name="x_tile",
                                     tag="x_tile")
                nc.sync.dma_start(x_tile[:, :, :width],
                                  XT_r[:, :, off:off + width])
                for s in range(width // P):
                    ib = off // P + s
                    psum_full = psum_pool.tile([P, 512], F32, name="psum_h",
                                               tag="psum")
                    psum_t = psum_full[:, :d_out]
                    for a in range(DB):
                        nc.tensor.matmul(
                            psum_t,
                            lhsT=x_tile[:, a, s * P:(s + 1) * P],
                            rhs=w_blk[a][:],
                            start=(a == 0),
                            stop=(a == DB - 1),
                        )
                    nc.vector.tensor_add(
                        out=h_all[:, ib, :], in0=psum_t, in1=b128[:])

            # ---- phase 2: aggregation outT = (A_rows @ H)^T ----
            # Two sequential column groups: group 0 covers out rows
            # [0, rows/2), group 1 the rest. Group 0's writeback overlaps
            # group 1's compute, hiding half the tail.
            n_grp = 2 if N_IC % 2 == 0 else 1
            ic_per_grp = N_IC // n_grp
            gw = ic_per_grp * NC_F  # A columns per group
            for g in range(n_grp):
                psum_o = [
                    psum_pool.tile([P, NC_F], F32, name=f"psum_o{g}_{i}",
                                   tag="psum")
                    for i in range(JH * ic_per_grp)
                ]
                for kb in range(KB):
                    a_tile = a_pool.tile([P, gw], adt, name="a_tile",
                                         tag="a_tile")
                    nc.sync.dma_start(
                        a_tile[:],
                        AT[kb * P:(kb + 1) * P, g * gw:(g + 1) * gw])
                    for jh in range(JH):
                        lhsT = h_all[:, kb, jh * P:(jh + 1) * P]
                        for ic in range(ic_per_grp):
                            nc.tensor.matmul(
                                psum_o[jh * ic_per_grp + ic],
                                lhsT=lhsT,
                                rhs=a_tile[:, ic * NC_F:(ic + 1) * NC_F],
                                start=(kb == 0),
                                stop=(kb == KB - 1),
                            )
                # writeback of this group (overlaps next group's compute)
                for jh in range(JH):
                    for ic in range(ic_per_grp):
                        o_tile = o_pool.tile([P, NC_F], F32, name="o_tile",
                                             tag="o_tile")
                        if (jh * ic_per_grp + ic) % 2 == 0:
                            nc.vector.tensor_copy(
                                out=o_tile[:],
                                in_=psum_o[jh * ic_per_grp + ic][:])
                        else:
                            nc.scalar.copy(
                                out=o_tile[:],
                                in_=psum_o[jh * ic_per_grp + ic][:])
                        nc.sync.dma_start(
                            outT[jh * P:(jh + 1) * P,
                                 g * gw + ic * NC_F:g * gw + (ic + 1) * NC_F],
                            o_tile[:],
                        )

    nc.compile()
    return nc


def _prep_in_maps(X, A_hat, W, b, n_cores=N_CORES,
                  a_dtype=A_DTYPE, x_dtype=X_DTYPE):
    rows = A_hat.shape[0] // n_cores
    a_np = _np_dt(a_dtype)
    x_np = _np_dt(x_dtype)
    XT = np.ascontiguousarray(X.T).astype(x_np)
    Wx = np.ascontiguousarray(W).astype(x_np)
    b2 = np.ascontiguousarray(
        np.asarray(b).reshape(1, -1).astype(np.float32, copy=False))
    in_maps = []
    for c in range(n_cores):
        ATc = np.ascontiguousarray(
            A_hat[c * rows:(c + 1) * rows, :].T).astype(a_np)
        in_maps.append({"AT": ATc, "XT": XT, "W": Wx, "b": b2})
    return in_maps


def kernel(X, A_hat, W, b):
    X = np.asarray(X)
    A_hat = np.asarray(A_hat)
    W = np.asarray(W)
    b = np.asarray(b)
    in_maps = _prep_in_maps(X, A_hat, W, b)
    nc = build_gcn_nc()
    res = run_bass_kernel_spmd(nc, in_maps, core_ids=list(range(N_CORES)))
    out = np.concatenate(
        [np.asarray(r["outT"]).T for r in res.results], axis=0)
    return np.ascontiguousarray(out.astype(np.float32, copy=False))


# revision 21
# speedup vs baseline: 1.0078x; 1.0078x over previous
"""GCNConv kernel for 8 TRN2 NeuronCores.

Computes: out = A_hat @ (X @ W + b)
  X: [16384, 512] f32   A_hat: [16384, 16384] f32
  W: [512, 256] f32     b: [256] f32          out: [16384, 256] f32

Sharding: row-shard A_hat / out across 8 cores (2048 rows each); replicate
X, W, b. Each core computes the full projection H = X @ W + b (SBUF-
resident) and then its slice of the aggregation A_rows @ H.

Host-side layout prep (sharding, not device work):
  - A shard is pre-transposed to AT = A[rows, :].T -> [16384, 2048] so the
    contraction dim lands on SBUF partitions with coalesced DMA loads, and
    converted to bf16 (halves HBM traffic; fp32 accumulation on device).
  - X is pre-transposed to XT = X.T -> [512, 16384] (bf16).
The device output is outT = (A_rows @ H).T [256, 2048]; the host
transposes back and concatenates. All accumulation is fp32 in PSUM.
"""

import numpy as np
import ml_dtypes

import concourse.bass as bass
import concourse.mybir as mybir
import concourse.tile as tile
from concourse import bacc
from concourse.bass_utils import run_bass_kernel_spmd

N = 16384
D_IN = 512
D_OUT = 256
N_CORES = 8
ROWS = N // N_CORES  # 2048 A/out rows per core

P = 128
F32 = mybir.dt.float32
F32R = mybir.dt.float32r
BF16 = mybir.dt.bfloat16

# compute dtype config: 'bf16' or 'f32r' for the A/X/W/H operand storage
A_DTYPE = "bf16"
X_DTYPE = "bf16"


def _dt(tag):
    return BF16 if tag == "bf16" else F32R


def _np_dt(tag):
    return ml_dtypes.bfloat16 if tag == "bf16" else np.float32


def build_gcn_nc(n=N, d_in=D_IN, d_out=D_OUT, rows=ROWS,
                 a_dtype=A_DTYPE, x_dtype=X_DTYPE, a_bufs=12, x_cols=512):
    """Per-core SPMD program.

    DRAM params (per core):
      AT  [n, rows]    a_dtype  - A shard, transposed
      XT  [d_in, n]    x_dtype  - X, transposed (replicated)
      W   [d_in, d_out] x_dtype
      b   [1, d_out]   f32
      outT [d_out, rows] f32 (output)
    """
    KB = n // P            # aggregation contraction blocks (128)
    DB = d_in // P         # projection contraction blocks (4)
    XC = min(x_cols, n)    # X columns loaded per DMA
    XB = n // XC           # X load blocks (32)
    XS = XC // P           # i-subblocks per X load (4)
    JH = d_out // P        # output-column halves (2)
    NC_F = min(512, rows)
    N_IC = rows // NC_F    # rhs chunks per aggregation step (4)

    adt = _dt(a_dtype)
    xdt = _dt(x_dtype)

    nc = bacc.Bacc("TRN2", target_bir_lowering=False, debug=False,
                   num_devices=N_CORES)

    AT = nc.dram_tensor("AT", [n, rows], adt, kind="ExternalInput").ap()
    XT = nc.dram_tensor("XT", [d_in, n], xdt, kind="ExternalInput").ap()
    W = nc.dram_tensor("W", [d_in, d_out], xdt, kind="ExternalInput").ap()
    b = nc.dram_tensor("b", [1, d_out], F32, kind="ExternalInput").ap()
    outT = nc.dram_tensor("outT", [d_out, rows], F32, kind="ExternalOutput").ap()

    XT_r = XT.rearrange("(a p) i -> p a i", p=P)   # [128, DB, n]
    W_r = W.rearrange("(a p) j -> p a j", p=P)     # [128, DB, d_out]

    with tile.TileContext(nc) as tc:
        with (
            tc.tile_pool(name="const", bufs=1) as const_pool,
            tc.tile_pool(name="hbuf", bufs=1) as h_pool,
            tc.tile_pool(name="xbuf", bufs=6) as x_pool,
            tc.tile_pool(name="abuf", bufs=a_bufs) as a_pool,
            tc.tile_pool(name="obuf", bufs=2) as o_pool,
            tc.tile_pool(name="psum", bufs=8, space="PSUM") as psum_pool,
        ):
            # ---- constants ----
            # per-k-block W tiles so the first matmul starts after 1/DB of
            # the W load
            w_blk = [const_pool.tile([P, d_out], xdt, name=f"w_blk{a}")
                     for a in range(DB)]
            for a in range(DB):
                nc.sync.dma_start(w_blk[a][:], W_r[:, a, :])
            b_sb = const_pool.tile([1, d_out], F32)
            nc.sync.dma_start(b_sb[:], b[:])
            b128 = const_pool.tile([P, d_out], F32)
            nc.gpsimd.partition_broadcast(b128[:], b_sb[:])

            # H = X @ W + b, stored as h_all[p, kb, j] = H[kb*128 + p, j]
            h_all = h_pool.tile([P, KB, d_out], adt)

            # ---- phase 1: projection ----
            # (first block loads narrow so the PE starts as early as possible)
            blocks = [(0, P)] + [(P + j * XC, XC) for j in range((n - P) // XC)]
            rem = (n - P) % XC
            if rem:
                blocks.append((n - rem, rem))
            for off, width in blocks:
                x_tile = x_pool.tile([P, DB, XC], xdt, name="x_tile",
                                     tag="x_tile")
                nc.sync.dma_start(x_tile[:, :, :width],
                                  XT_r[:, :, off:off + width])
                for s in range(width // P):
                    ib = off // P + s
                    psum_full = psum_pool.tile([P, 512], F32, name="psum_h",
                                               tag="psum")
                    psum_t = psum_full[:, :d_out]
                    for a in range(DB):
                        nc.tensor.matmul(
                            psum_t,
                            lhsT=x_tile[:, a, s * P:(s + 1) * P],
                            rhs=w_blk[a][:],
                            start=(a == 0),
                            stop=(a == DB - 1),
                        )
                    nc.vector.tensor_add(
                        out=h_all[:, ib, :], in0=psum_t, in1=b128[:])

            # ---- phase 2: aggregation outT = (A_rows @ H)^T ----
            # Two sequential column groups: group 0 covers out rows
            # [0, rows/2), group 1 the rest. Group 0's writeback overlaps
            # group 1's compute, hiding half the tail.
            n_grp = 2 if N_IC % 2 == 0 else 1
            ic_per_grp = N_IC // n_grp
            gw = ic_per_grp * NC_F  # A columns per group
            for g in range(n_grp):
                psum_o = [
                    psum_pool.tile([P, NC_F], F32, name=f"psum_o{g}_{i}",
                                   tag="psum")
                    for i in range(JH * ic_per_grp)
                ]
                for kb in range(KB):
                    a_tile = a_pool.tile([P, gw], adt, name="a_tile",
                                         tag="a_tile")
                    nc.sync.dma_start(
                        a_tile[:],
                        AT[kb * P:(kb + 1) * P, g * gw:(g + 1) * gw])
                    for jh in range(JH):
                        lhsT = h_all[:, kb, jh * P:(jh + 1) * P]
                        for ic in range(ic_per_grp):
                            nc.tensor.matmul(
                                psum_o[jh * ic_per_grp + ic],
                                lhsT=lhsT,
                                rhs=a_tile[:, ic * NC_F:(ic + 1) * NC_F],
                                start=(kb == 0),
                                stop=(kb == KB - 1),
                            )
                # writeback of this group (overlaps next group's compute)
                for jh in range(JH):
                    for ic in range(ic_per_grp):
                        o_tile = o_pool.tile([P, NC_F], F32, name="o_tile",
                                             tag="o_tile")
                        if (jh * ic_per_grp + ic) % 2 == 0:
                            nc.vector.tensor_copy(
                                out=o_tile[:],
                                in_=psum_o[jh * ic_per_grp + ic][:])
                        else:
                            nc.scalar.copy(
                                out=o_tile[:],
                                in_=psum_o[jh * ic_per_grp + ic][:])
                        nc.sync.dma_start(
                            outT[jh * P:(jh + 1) * P,
                                 g * gw + ic * NC_F:g * gw + (ic + 1) * NC_F],
                            o_tile[:],
                        )

    nc.compile()
    return nc


def _prep_in_maps(X, A_hat, W, b, n_cores=N_CORES,
                  a_dtype=A_DTYPE, x_dtype=X_DTYPE):
    rows = A_hat.shape[0] // n_cores
    a_np = _np_dt(a_dtype)
    x_np = _np_dt(x_dtype)
    XT = np.ascontiguousarray(X.T).astype(x_np)
    Wx = np.ascontiguousarray(W).astype(x_np)
    b2 = np.ascontiguousarray(
        np.asarray(b).reshape(1, -1).astype(np.float32, copy=False))
    in_maps = []
    for c in range(n_cores):
        ATc = np.ascontiguousarray(
            A_hat[c * rows:(c + 1) * rows, :].T).astype(a_np)
        in_maps.append({"AT": ATc, "XT": XT, "W": Wx, "b": b2})
    return in_maps


def kernel(X, A_hat, W, b):
    X = np.asarray(X)
    A_hat = np.asarray(A_hat)
    W = np.asarray(W)
    b = np.asarray(b)
    in_maps = _prep_in_maps(X, A_hat, W, b)
    nc = build_gcn_nc()
    res = run_bass_kernel_spmd(nc, in_maps, core_ids=list(range(N_CORES)))
    out = np.concatenate(
        [np.asarray(r["outT"]).T for r in res.results], axis=0)
    return np.ascontiguousarray(out.astype(np.float32, copy=False))
